# revision 1
# baseline (speedup 1.0000x reference)
"""Trainium2 Bass kernel v2 for nn_Attention_43894565765586.

Sharding: 8 cores = (batch b in {0,1}) x (sub-head group g in {0..3}).
Core (b,g) owns sub-heads S_g = {4g..4g+3} of the 16 shared K sub-heads,
for ALL 4 branches -> 16 (br,h) heads per core. Benefits vs per-branch:
  - K projection is 4x smaller (only 4 sub-head column blocks of WK).
  - mean over branches commutes with WO: each core sums its branches'
    ctx locally and applies a [256,1024] slice of WO (4x less matmul);
    host sums the 4 partial y's per batch and adds bO.
  - banded-block pruning is load-balanced: every core does the same
    67-block schedule instead of the worst branch's schedule.

Off-band softmax trick (as baseline): off-band logits are 0 (exp=1), so
  numer = V_sum + sum_band (exp(w)-1) v + e_sink*v_null
  Z     = T + sum_band (exp(w)-1) + e_sink
V_sum/extras are computed on host (xsum @ WV slice) and uploaded.

Score-block schedule per (sub-head h, query block qb): offsets o=qb-kb
valid per branch: br0/1/2 (bands within 128): o in {0,1}; br3 (band
[144,512)): o in {1,2,3,4}. Blocks live in one PSUM tile [128,1536] f32
(3 banks) at o-major slots:
  bank0: [0:128) br0@o0 [128:256) br1@o0 [256:384) br2@o0, gap [384:512)
  bank1: [512:640) br0@o1(16 q-cols) [640:768) br1@o1 [768:896) br2@o1
         [896:1024) br3@o1
  bank2: [1024:1152) br3@o2 [1152:1280) br3@o3 [1280:1408) br3@o4
exp/stt run per-bank so the next sub-head's scores can pipeline.
"""

import sys
from contextlib import ExitStack

import numpy as np

sys.path.insert(0, "/opt/trn_rl_repo")

import concourse.bass as bass
import concourse.tile as tile
from concourse import bacc, mybir
from concourse.bass_utils import run_bass_kernel_spmd


def _install_ntff_shim():
    """Provide antenv.axon_hooks (missing in this image) so BASS_TRACE
    profiling works: drives NTFF capture via libaxon_pjrt.so ctypes."""
    import types, ctypes, contextlib

    try:
        from antenv.axon_hooks import get_axon_ntff_profile_hook  # noqa
        return  # real module present
    except ImportError:
        pass
    try:
        import antenv
    except ImportError:
        return
    mod = types.ModuleType("antenv.axon_hooks")
    mod._hook = None

    def set_axon_ntff_profile_hook(h):
        mod._hook = h

    def get_axon_ntff_profile_hook():
        return mod._hook

    mod.set_axon_ntff_profile_hook = set_axon_ntff_profile_hook
    mod.get_axon_ntff_profile_hook = get_axon_ntff_profile_hook
    sys.modules["antenv.axon_hooks"] = mod
    antenv.axon_hooks = mod

    so_path = "/opt/axon/libaxon_pjrt.so"
    try:
        lib = ctypes.CDLL(so_path)
        if not hasattr(lib, "axon_start_nrt_profile"):
            return
        lib.axon_start_nrt_profile.argtypes = [
            ctypes.POINTER(ctypes.c_int64), ctypes.c_size_t]
        lib.axon_start_nrt_profile.restype = ctypes.c_int64
        lib.axon_stop_nrt_profile.argtypes = [ctypes.c_char_p]
        lib.axon_stop_nrt_profile.restype = ctypes.c_int64
    except OSError:
        return

    @contextlib.contextmanager
    def _hook(output_dir, device_ids):
        import jax
        jax.devices()
        if device_ids:
            ids = (ctypes.c_int64 * len(device_ids))(*device_ids)
            rc = lib.axon_start_nrt_profile(ids, len(device_ids))
        else:
            rc = lib.axon_start_nrt_profile(None, 0)
        if rc != 0:
            raise RuntimeError(f"axon_start_nrt_profile rc={rc}")
        try:
            yield
        finally:
            n = lib.axon_stop_nrt_profile(str(output_dir).encode())
            print(f"ntff profile: {n} file(s) -> {output_dir}", file=sys.stderr)

    set_axon_ntff_profile_hook(_hook)


_install_ntff_shim()

import ml_dtypes
F16 = mybir.dt.float16
NPF16 = np.float16
F32 = mybir.dt.float32

D_MODEL = 1024
N_SH = 16
N_BR = 4
H_TOT = 64
DH = 64
T = 1024
B = 2
NT = T // 128
NK = D_MODEL // 128
BRANCH_CONFIGS = [(16, 0), (128, 16), (128, 0), (512, 144)]
EPS = float(np.finfo(np.float32).eps)

Exp = mybir.ActivationFunctionType.Exp
Sqrt = mybir.ActivationFunctionType.Sqrt
Square = mybir.ActivationFunctionType.Square
Identity = mybir.ActivationFunctionType.Identity
ADD = mybir.AluOpType.add
MULT = mybir.AluOpType.mult

# phase-1 scores tile [128,1024] (2 PSUM banks) slot layout; phase-2 tile
# holds br3 o=3,4 at cols 0/128 (mask const cols 1024+). br0-o1 is 16 wide.
SLOT = {(0, 0): 0, (1, 0): 128, (2, 0): 256, (3, 2): 384,
        (0, 1): 512, (1, 1): 640, (2, 1): 768, (3, 1): 896}
SLOTP2 = {(3, 3): 0, (3, 4): 128}
WIDTH = {k: (16 if k == (0, 1) else 128) for k in list(SLOT) + list(SLOTP2)}


def _ap(t, offset_delta, dims):
    return bass.AP(tensor=t.tensor, offset=t.offset + offset_delta, ap=list(dims))


def _build_nc():
    nc = bacc.Bacc("TRN2", target_bir_lowering=False, debug=False, num_devices=8)

    xt = nc.dram_tensor("xt", [D_MODEL, T], F16, kind="ExternalInput")
    wq = nc.dram_tensor("wq", [D_MODEL, 1024], F16, kind="ExternalInput")
    wk = nc.dram_tensor("wk", [D_MODEL, 256], F16, kind="ExternalInput")
    wv = nc.dram_tensor("wv", [D_MODEL, 1024], F16, kind="ExternalInput")
    wo = nc.dram_tensor("wo", [384, 1024], F16, kind="ExternalInput")
    bq = nc.dram_tensor("bq", [1, 1024], F16, kind="ExternalInput")
    bk = nc.dram_tensor("bk", [1, 256], F16, kind="ExternalInput")
    bv = nc.dram_tensor("bv", [1, 1024], F16, kind="ExternalInput")
    trig1 = nc.dram_tensor("trig1", [T, DH], F16, kind="ExternalInput")
    trig2 = nc.dram_tensor("trig2", [T, DH], F16, kind="ExternalInput")
    masks = nc.dram_tensor("masks", [128, 1280], F16, kind="ExternalInput")
    zc = nc.dram_tensor("zc", [1, 16], F32, kind="ExternalInput")

    yt = nc.dram_tensor("yt", [1024, T], F32, kind="ExternalOutput")

    with tile.TileContext(nc) as tc, ExitStack() as top:
        const = top.enter_context(tc.tile_pool(name="const", bufs=1))
        persist = top.enter_context(tc.tile_pool(name="persist", bufs=1))

        wo_sb = const.tile([128, 3, 1024], F16)
        trig1_sb = const.tile([128, NT, DH], F16)
        nc.sync.dma_start(trig1_sb[:], trig1.ap().rearrange("(tt tp) j -> tp tt j", tp=128))
        trig2_sb = const.tile([128, NT, DH], F16)
        nc.sync.dma_start(trig2_sb[:], trig2.ap().rearrange("(tt tp) j -> tp tt j", tp=128))
        mask_sb = const.tile([128, 1280], F16)
        zc_sb = const.tile([128, 16], F32)
        nc.sync.dma_start(zc_sb[:], _ap(zc.ap(), 0, [[0, 128], [1, 16]]))
        ones1 = const.tile([1, 128], F16)
        nc.vector.memset(ones1[:], 1.0)
        eps_q = const.tile([128, 1], F32)
        nc.vector.memset(eps_q[:], EPS)
        zero_b = const.tile([128, 1], F32)
        nc.vector.memset(zero_b[:], 0.0)

        qT_sb = persist.tile([128, NK, T], F16)      # Q^T feature-major
        kT_sb = persist.tile([128, 2, T], F16)       # K^T feature-major (4 sub-heads)
        v_sb = persist.tile([128, NT, 16, 65], F16)  # V token-major + ones col
        ctxT_sb = persist.tile([128, 3, T], F16)     # [ctx_sum(256); zinv(16); pad]^T

        # ---- stage B: projections + norm + rope ----
        with ExitStack() as stage_b:
            wts = stage_b.enter_context(tc.tile_pool(name="wts", bufs=1))
            pj = stage_b.enter_context(tc.tile_pool(name="pj", bufs=6, space="PSUM"))
            work = stage_b.enter_context(tc.tile_pool(name="work", bufs=3))
            work3 = stage_b.enter_context(tc.tile_pool(name="work3", bufs=7))
            stats = stage_b.enter_context(tc.tile_pool(name="stats", bufs=6))

            xt_sb = wts.tile([128, NK, T], F16)
            nc.sync.dma_start(xt_sb[:], xt.ap().rearrange("(kt kp) t -> kp kt t", kp=128))
            wq_sb = wts.tile([128, NK, 1024], F16)
            nc.sync.dma_start(wq_sb[:], wq.ap().rearrange("(kt kp) n -> kp kt n", kp=128))
            wk_sb = wts.tile([128, NK, 256], F16)
            nc.sync.dma_start(wk_sb[:], wk.ap().rearrange("(kt kp) n -> kp kt n", kp=128))
            wv_sb = wts.tile([128, NK, 1024], F16)
            nc.sync.dma_start(wv_sb[:], wv.ap().rearrange("(kt kp) n -> kp kt n", kp=128))
            bq_sb = wts.tile([1, 1024], F16)
            nc.sync.dma_start(bq_sb[:], bq.ap())
            bk_sb = wts.tile([1, 256], F16)
            nc.sync.dma_start(bk_sb[:], bk.ap())
            bv_sb = wts.tile([1, 1024], F16)
            nc.sync.dma_start(bv_sb[:], bv.ap())
            nc.sync.dma_start(mask_sb[:], masks.ap())
            nc.sync.dma_start(wo_sb[:], wo.ap().rearrange("(kt kp) n -> kp kt n", kp=128))

            def project(tt, w_sb, b_sb, chunks):
                """X @ W + b for token tile tt -> PSUM chunks [(col0, n)]."""
                out = []
                for (c0, n) in chunks:
                    ps = pj.tile([128, 512], F32, tag="pj")
                    for kt in range(NK):
                        nc.tensor.matmul(
                            ps[:, :n], xt_sb[:, kt, tt * 128:(tt + 1) * 128],
                            w_sb[:, kt, c0:c0 + n],
                            start=(kt == 0), stop=False)
                    nc.tensor.matmul(ps[:, :n], ones1[:], b_sb[:, c0:c0 + n],
                                     start=False, stop=True)
                    out.append(ps)
                return out

            def norm_rope(tt, halves, ncols, is_q, outT_sb):
                nh = ncols // 64          # heads in this tensor
                wtag = "f16w" if ncols == 1024 else "f16k"
                x16 = work3.tile([128, ncols], F16, tag=wtag)
                with nc.allow_low_precision(reason="activations f16"):
                    for i, (c0, n) in enumerate(
                            [(0, 512), (512, 512)] if ncols == 1024 else [(0, 256)]):
                        nc.scalar.copy(x16[:, c0:c0 + n], halves[i][:, :n])
                sq = work3.tile([128, ncols], F16, tag=wtag)
                with nc.allow_low_precision(reason="squares f16"):
                    nc.scalar.activation(sq[:], x16[:], Square)
                ss = stats.tile([128, nh], F32, tag="ss")
                nc.vector.tensor_reduce(
                    ss[:], _ap(sq[:], 0, [sq[:].ap[0], [64, nh], [1, 64]]),
                    axis=mybir.AxisListType.X, op=ADD)
                rt = stats.tile([128, nh], F32, tag="rt")
                if is_q:
                    nc.scalar.activation(rt[:], ss[:], Sqrt, bias=eps_q[:], scale=1.0 / DH)
                else:
                    nc.scalar.activation(rt[:], ss[:], Sqrt, bias=zero_b[:], scale=1.0)
                rs = stats.tile([128, nh], F32, tag="rs")
                nc.vector.reciprocal_approx_fast(rs[:], rt[:])
                rs16 = stats.tile([128, nh], F16, tag="rs16")
                with nc.allow_low_precision(reason="scale f16"):
                    nc.vector.tensor_copy(rs16[:], rs[:])

                p1 = work3.tile([128, ncols], F16, tag=wtag)
                a1 = _ap(x16[:], 0, [x16[:].ap[0], [64, nh], [0, 2], [1, 32]])
                tb1 = _ap(trig1_sb[:, tt, :], 0,
                          [trig1_sb[:, tt, :].ap[0], [0, nh], [32, 2], [1, 32]])
                p2 = work3.tile([128, ncols], F16, tag=wtag)
                a2 = _ap(x16[:], 32, [x16[:].ap[0], [64, nh], [0, 2], [1, 32]])
                tb2 = _ap(trig2_sb[:, tt, :], 0,
                          [trig2_sb[:, tt, :].ap[0], [0, nh], [32, 2], [1, 32]])
                sm = work3.tile([128, ncols], F16, tag=wtag)
                xn = work3.tile([128, ncols], F16, tag=wtag)
                with nc.allow_low_precision(reason="rope in f16"):
                    nc.gpsimd.tensor_tensor(p1[:], a1, tb1, op=MULT)
                    nc.gpsimd.tensor_tensor(p2[:], a2, tb2, op=MULT)
                    nc.vector.scalar_tensor_tensor(sm[:], p1[:], 0.0, p2[:],
                                                   op0=ADD, op1=ADD)
                    nc.vector.scalar_tensor_tensor(
                        xn[:], sm[:], 0.0,
                        _ap(rs16[:], 0, [rs16[:].ap[0], [1, nh], [0, 64]]),
                        op0=ADD, op1=MULT)
                nc.sync.dma_start_transpose(
                    outT_sb[:, :, tt * 128:(tt + 1) * 128], xn[:])

            for tt in range(NT):
                norm_rope(tt, project(tt, wq_sb, bq_sb, [(0, 512), (512, 512)]),
                          1024, True, qT_sb)
                norm_rope(tt, project(tt, wk_sb, bk_sb, [(0, 256)]),
                          256, False, kT_sb)
                vh = project(tt, wv_sb, bv_sb, [(0, 512), (512, 512)])
                with nc.allow_low_precision(reason="v f16"):
                    for i in range(2):
                        out = _ap(v_sb[:, tt, :, :], i * 8 * 65,
                                  [v_sb[:, tt, :, :].ap[0], [65, 8], [1, 64]])
                        nc.scalar.copy(out, vh[i][:])
                nc.vector.memset(
                    _ap(v_sb[:, tt, :, :], 64, [v_sb[:, tt, :, :].ap[0], [65, 16], [1, 1]]),
                    1.0)

        # ---- stage D: banded attention ----
        with ExitStack() as stage_d:
            spool = stage_d.enter_context(tc.tile_pool(name="spool", bufs=2, space="PSUM"))
            cpool = stage_d.enter_context(tc.tile_pool(name="cpool", bufs=2, space="PSUM"))
            att = stage_d.enter_context(tc.tile_pool(name="att", bufs=3))
            est = stage_d.enter_context(tc.tile_pool(name="est", bufs=4))

            for qb in range(NT):
                ctxm = att.tile([128, 384], F16, tag="ctxm")
                for hh in range(2):
                    ctx_ps = cpool.tile([128, 1024], F32, tag="ctx")
                    if qb == 0:
                        nc.vector.memset(
                            _ap(ctx_ps[:], 384, [ctx_ps[:].ap[0], [512, 2], [1, 128]]), 0.0)
                    for hl in range(2):
                        h = 2 * hh + hl
                        p0 = (h % 2) * 64
                        ft = h // 2
                        vend = 384 if qb == 0 else 1024
                        s_ps = spool.tile([128, 1024], F32, tag="s")
                        q_base = qT_sb[p0:p0 + 64, ft, qb * 128:(qb + 1) * 128]
                        nc.tensor.matmul(
                            s_ps[:, 0:384],
                            kT_sb[p0:p0 + 64, ft, qb * 128:(qb + 1) * 128],
                            _ap(q_base, 0, [q_base.ap[0], [2048, 3], [1, 128]]),
                            start=True, stop=True)
                        if qb >= 1:
                            kb = qb - 1
                            nc.tensor.matmul(
                                s_ps[:, 512:528],
                                kT_sb[p0:p0 + 64, ft, kb * 128:(kb + 1) * 128],
                                qT_sb[p0:p0 + 64, ft, qb * 128:qb * 128 + 16],
                                start=True, stop=True)
                            q_base2 = qT_sb[p0:p0 + 64, ft + 2, qb * 128:(qb + 1) * 128]
                            nc.tensor.matmul(
                                s_ps[:, 640:1024],
                                kT_sb[p0:p0 + 64, ft, kb * 128:(kb + 1) * 128],
                                _ap(q_base2, 0, [q_base2.ap[0], [2048, 3], [1, 128]]),
                                start=True, stop=True)
                        if qb >= 2:
                            nc.tensor.matmul(
                                s_ps[:, 384:512],
                                kT_sb[p0:p0 + 64, ft, (qb - 2) * 128:(qb - 1) * 128],
                                qT_sb[p0:p0 + 64, 6 + ft, qb * 128:(qb + 1) * 128],
                                start=True, stop=True)
                        e16 = est.tile([128, 1024], F16, tag="e")
                        s16 = est.tile([128, 1024], F16, tag="s16")
                        with nc.allow_low_precision(reason="exp scores f16"):
                            nc.scalar.activation(e16[:, :vend], s_ps[:, :vend], Exp)
                            nc.vector.scalar_tensor_tensor(
                                s16[:, :vend], e16[:, :vend], -1.0,
                                mask_sb[:, :vend], op0=ADD, op1=MULT)
                        if qb >= 3:
                            vend2 = 128 * min(qb - 2, 2)
                            s_ps2 = spool.tile([128, 1024], F32, tag="s")
                            nc.tensor.matmul(
                                s_ps2[:, 0:128],
                                kT_sb[p0:p0 + 64, ft, (qb - 3) * 128:(qb - 2) * 128],
                                qT_sb[p0:p0 + 64, 6 + ft, qb * 128:(qb + 1) * 128],
                                start=True, stop=True)
                            if qb >= 4:
                                nc.tensor.matmul(
                                    s_ps2[:, 128:256],
                                    kT_sb[p0:p0 + 64, ft, (qb - 4) * 128:(qb - 3) * 128],
                                    qT_sb[p0:p0 + 64, 6 + ft, qb * 128:(qb + 1) * 128],
                                    start=True, stop=True)
                            e2 = est.tile([128, 256], F16, tag="e2")
                            s162 = est.tile([128, 256], F16, tag="s162")
                            with nc.allow_low_precision(reason="exp scores f16"):
                                nc.scalar.activation(e2[:, :vend2], s_ps2[:, :vend2], Exp)
                                nc.vector.scalar_tensor_tensor(
                                    s162[:, :vend2], e2[:, :vend2], -1.0,
                                    mask_sb[:, 1024:1024 + vend2], op0=ADD, op1=MULT)
                        # PV: h-major ctx col j = hl*4+br within this half tile
                        for br in range(4):
                            jcol = (hl * 4 + br) * 128
                            if br < 3:
                                offs = [0, 1] if qb >= 1 else [0]
                                for i, o in enumerate(offs):
                                    c0 = SLOT[(br, o)]
                                    w = WIDTH[(br, o)]
                                    nc.tensor.matmul(
                                        ctx_ps[:w, jcol:jcol + 65],
                                        s16[:, c0:c0 + w],
                                        v_sb[:, qb - o, hl * 4 + br + 8 * hh, :],
                                        start=(i == 0), stop=(i == len(offs) - 1))
                            else:
                                offs = [o for o in (1, 2) if qb - o >= 0]
                                offs2 = [o for o in (3, 4) if qb - o >= 0]
                                for i, o in enumerate(offs):
                                    nc.tensor.matmul(
                                        ctx_ps[:, jcol:jcol + 65],
                                        s16[:, SLOT[(3, o)]:SLOT[(3, o)] + 128],
                                        v_sb[:, qb - o, hl * 4 + 3 + 8 * hh, :],
                                        start=(i == 0),
                                        stop=(i == len(offs) - 1 and not offs2))
                                for i, o in enumerate(offs2):
                                    nc.tensor.matmul(
                                        ctx_ps[:, jcol:jcol + 65],
                                        s162[:, SLOTP2[(3, o)]:SLOTP2[(3, o)] + 128],
                                        v_sb[:, qb - o, hl * 4 + 3 + 8 * hh, :],
                                        start=False, stop=(i == len(offs2) - 1))
                    # epilogue for this half (8 heads j = hh*8 + h-major idx)
                    zz = att.tile([128, 8], F32, tag="zz")
                    nc.vector.scalar_tensor_tensor(
                        zz[:], _ap(ctx_ps[:], 64, [ctx_ps[:].ap[0], [128, 8]]),
                        0.0, zc_sb[:, hh * 8:hh * 8 + 8], op0=ADD, op1=ADD)
                    zinv = att.tile([128, 8], F32, tag="zinv")
                    nc.vector.reciprocal_approx_fast(zinv[:], zz[:])
                    ctxn = att.tile([128, 512], F16, tag="ctxn")
                    with nc.allow_low_precision(reason="ctx f16"):
                        nc.vector.scalar_tensor_tensor(
                            ctxn[:], _ap(ctx_ps[:], 0, [ctx_ps[:].ap[0], [128, 8], [1, 64]]),
                            0.0, _ap(zinv[:], 0, [zinv[:].ap[0], [1, 8], [0, 64]]),
                            op0=ADD, op1=MULT)
                        uA = att.tile([128, 128], F16, tag="uA")
                        uB = att.tile([128, 128], F16, tag="uB")
                        nc.vector.scalar_tensor_tensor(
                            uA[:], _ap(ctxn[:], 0, [ctxn[:].ap[0], [256, 2], [1, 64]]),
                            0.0, _ap(ctxn[:], 64, [ctxn[:].ap[0], [256, 2], [1, 64]]),
                            op0=ADD, op1=ADD)
                        nc.gpsimd.tensor_tensor(
                            uB[:], _ap(ctxn[:], 128, [ctxn[:].ap[0], [256, 2], [1, 64]]),
                            _ap(ctxn[:], 192, [ctxn[:].ap[0], [256, 2], [1, 64]]), op=ADD)
                        nc.vector.scalar_tensor_tensor(
                            ctxm[:, hh * 128:hh * 128 + 128], uA[:], 0.0, uB[:],
                            op0=ADD, op1=ADD)
                        nc.vector.tensor_copy(ctxm[:, 256 + hh * 8:256 + hh * 8 + 8], zinv[:])
                nc.vector.memset(ctxm[:, 272:384], 0.0)
                nc.sync.dma_start_transpose(
                    ctxT_sb[:, :, qb * 128:(qb + 1) * 128], ctxm[:])

        # ---- stage E: y^T = WO'^T @ ctx^T (bO added on host) ----
        with ExitStack() as stage_e:
            pe_ps = stage_e.enter_context(tc.tile_pool(name="pe_ps", bufs=4, space="PSUM"))
            ypool = stage_e.enter_context(tc.tile_pool(name="ypool", bufs=2))
            for co in range(8):
                y_sb = ypool.tile([128, 1024], F32, tag="y")
                for nh in range(2):
                    ps = pe_ps.tile([128, 512], F32, tag="y")
                    for kt in range(3):
                        nc.tensor.matmul(
                            ps[:], wo_sb[:, kt, co * 128:(co + 1) * 128],
                            ctxT_sb[:, kt, nh * 512:(nh + 1) * 512],
                            start=(kt == 0), stop=(kt == 2))
                    nc.scalar.copy(y_sb[:, nh * 512:(nh + 1) * 512], ps[:])
                nc.sync.dma_start(yt.ap()[co * 128:(co + 1) * 128, :], y_sb[:])

    nc.compile()
    return nc


_NC = None


def _get_nc():
    global _NC
    if _NC is None:
        _NC = _build_nc()
    return _NC


def _host_inputs(X, WQ, bQ, WK, bK, WV, bV, WO, bO, sink_scalars, v_nulls):
    dperm = np.concatenate([np.arange(0, 64, 2), np.arange(1, 64, 2)])

    inv_freq = 1.0 / (10000.0 ** (np.arange(0, DH, 2, dtype=np.float64) / DH))
    freqs = np.arange(T, dtype=np.float64)[:, None] * inv_freq[None, :]
    cos, sin = np.cos(freqs), np.sin(freqs)
    trig1 = np.concatenate([cos, sin], axis=1).astype(NPF16)
    trig2 = np.concatenate([-sin, cos], axis=1).astype(NPF16)

    e_sink = np.exp(np.tanh(sink_scalars.astype(np.float64)))  # [64], idx = br*16+sh

    # masks [128, 1536]: slot layout per (br, o); mask[kr, qr]
    qr = np.arange(128)[None, :]
    kr = np.arange(128)[:, None]
    masks = np.zeros((128, 1280), dtype=NPF16)
    for (br, o), c0 in list(SLOT.items()) + [((3, 3), 1024), ((3, 4), 1152)]:
        hb, gb = BRANCH_CONFIGS[br]
        d = 128 * o + qr - kr
        m = ((d >= gb) & (d < hb)).astype(NPF16)
        masks[:, c0:c0 + WIDTH[(br, o)]] = m[:, :WIDTH[(br, o)]]

    xsum = X.astype(np.float64).sum(axis=1)  # [B, 1024]

    in_maps = []
    for c in range(8):
        b_idx, g = c // 4, c % 4
        sh = np.arange(4 * g, 4 * g + 4)          # global sub-head ids
        # column gather for WQ/WV: brh-major (br, h) blocks of 64
        qcols = np.concatenate([
            (br * N_SH + s) * 64 + dperm for br in range(N_BR) for s in sh])
        vcols = np.concatenate([
            (br * N_SH + s) * 64 + np.arange(64) for s in sh for br in range(N_BR)])
        kcols = np.concatenate([s * 64 + dperm for s in sh])
        worows = np.concatenate([s * 64 + np.arange(64) for s in sh])

        es = e_sink[[br * N_SH + s for s in sh for br in range(N_BR)]]  # [16]
        vn = v_nulls[[br * N_SH + s for s in sh for br in range(N_BR)]]  # [16,64]
        bv_c = bV[vcols].astype(np.float64).reshape(16, 64)
        xw = (xsum[b_idx] @ WV.astype(np.float64))[vcols].reshape(16, 64)
        extras = T * bv_c + es[:, None] * vn.astype(np.float64) + xw  # [16, 64]
        # wo_aug: rows 0:256 = WO[sub-head rows]/4; rows 256+brh = G[brh]
        # where G[brh] = extras[brh] @ WO[rows of sub-head h]/4
        wo_aug = np.zeros((384, 1024), dtype=np.float64)
        wo_aug[:256] = WO[worows].astype(np.float64) / N_BR
        for j in range(16):          # h-major: j = h_local*4 + br
            rows = worows[(j // 4) * 64:(j // 4) * 64 + 64]
            wo_aug[256 + j] = extras[j] @ (WO[rows].astype(np.float64) / N_BR)

        in_maps.append({
            "xt": np.ascontiguousarray(X[b_idx].T).astype(NPF16),
            "wq": WQ[:, qcols].astype(NPF16),
            "wk": WK[:, kcols].astype(NPF16),
            "wv": WV[:, vcols].astype(NPF16),
            "wo": wo_aug.astype(NPF16),
            "bq": bQ[qcols].reshape(1, 1024).astype(NPF16),
            "bk": bK[kcols].reshape(1, 256).astype(NPF16),
            "bv": bV[vcols].reshape(1, 1024).astype(NPF16),
            "trig1": trig1,
            "trig2": trig2,
            "masks": masks,
            "zc": (T + es).reshape(1, 16).astype(np.float32),
        })
    return in_maps


LAST_RESULTS = None


def kernel(**inputs):
    global LAST_RESULTS
    inputs = {k: np.asarray(v) for k, v in inputs.items()}
    nc = _get_nc()
    in_maps = _host_inputs(**inputs)
    res = run_bass_kernel_spmd(nc, in_maps, list(range(8)))
    LAST_RESULTS = res
    y = np.zeros((B, T, D_MODEL), dtype=np.float64)
    for c in range(8):
        b_idx = c // 4
        y[b_idx] += res.results[c]["yt"].T.astype(np.float64)
    y += inputs["bO"].astype(np.float64)
    return y.astype(np.float32)

